# revision 1
# baseline (speedup 1.0000x reference)
"""TRN2 Bass kernel for nn_LoRACuetLinear (equivariant LoRA linear).

Math: for each irrep block j (9 blocks of 192 features; block j uses irrep
k(j) in {0,1,2}), out_seg = seg @ W_eff[k] where
  W_eff[k] = pw_base * Wb[k] + SCALING * pw_base * pw_B * (WA[k] @ WB[k])
(the LoRA branch folds exactly into the base weight since everything is
linear).

Device strategy (8 cores, data-parallel over nodes):
  - Host transposes x to x_T [1792(pad), rows] per core so the contraction
    dim (mul/feature) lies on SBUF partitions; the device then runs
    weights-stationary matmuls out_T = W^T x_T with the moving dim = rows.
  - Default mode "f16x3": the host splits x and W into fp16 high/low pairs
    (x = x1 + x2, W = w1 + w2, each fp16 with 11-bit significands), and the
    device accumulates x1@w1 + x2@w1 + x1@w2 into fp32 PSUM.  fp16 products
    of the 11-bit halves are exact in the fp32 accumulator, so the result
    has full fp32 accuracy (~3e-7 absmax rel, measured); the dropped x2@w2
    term is ~2^-22.  fp16 matmuls run at 1 cyc/row on the PE with separate,
    overlappable LDWEIGHTS and keep the HAM clock at 2.4 GHz (float32/
    float32r matmuls run 4x slower and do not register as PE activity, which
    leaves the clock gated at 1.2 GHz - measured).
  - Total DMA bytes are the same as shipping fp32 x (two fp16 planes).
  - Weights are packed per 128-row output section into a block-diagonal
    [128, 32*128] layout so every matmul has M=128 at psum partition base 0
    (fp32-family matmuls cannot target high PE column groups on TRN2, and
    this also keeps all DMA transfers 128-partition aligned).
  - psum->sbuf copies run on the Scalar engine; host un-transposes the
    gathered per-core outputs.
  - Fallback modes kept for experiments: "f32r3" (float32r 3-pass with
    on-device DVE split) and "f32r1" (single-pass float32r, ~1e-4 rel).
"""

import sys

sys.path.insert(0, "/opt/trn_rl_repo")

import os
import numpy as np

import concourse.bass as bass
import concourse.tile as tile
from concourse import bacc, mybir
from concourse.bass_utils import run_bass_kernel_spmd
# ---- problem constants (hardcoded per contract) ----
MUL = 192
DIMS = (1, 3, 5)
RANK = 8
SCALING = 2.0
N_NODES = 50000
FEAT = MUL * sum(DIMS)  # 1728
NCORES = 8
ROWS = N_NODES // NCORES  # 6250
FPAD = 1792  # 14 * 128
NSEC = FPAD // 128  # 14
R = 352  # row-tile (moving dim); 6250 = 17*352 + 266 (all tiles >= 256)
RF16 = 512  # row-tile for the f16 path (smaller SBUF tiles allow 512)
MODE = os.environ.get("LORA_KERNEL_MODE", "f16x3")  # f16x3 | f32r3 | f32r1
BLK_IRREP = [0] + [1] * 3 + [2] * 5

_MASK11 = np.uint32(0xFFFFF000)  # keep sign+exp+11 mantissa bits


def _section_mms():
    """Enumerate matmuls as (section, chunk, r0, r1, windex).

    Section s covers padded output rows [128s, 128s+128); chunk c covers
    padded input rows [128c, 128c+128).  (s, c) participates iff the
    block-diagonal weight has support there; r0:r1 is the nonzero input-row
    range within the chunk (always base 0 or 64, size 64 or 128).
    """
    sup = np.zeros((FPAD, FPAD), dtype=bool)
    for j in range(sum(DIMS)):
        sup[192 * j : 192 * j + 192, 192 * j : 192 * j + 192] = True
    mms = []
    wi = 0
    for s in range(NSEC):
        for c in range(NSEC):
            sl = sup[128 * c : 128 * c + 128, 128 * s : 128 * s + 128]
            nz = np.nonzero(sl.any(axis=1))[0]
            if len(nz) == 0:
                continue
            r0 = (int(nz[0]) // 64) * 64
            r1 = ((int(nz[-1]) + 64) // 64) * 64
            mms.append((s, c, r0, r1, wi))
            wi += 1
    return mms


_MMS = _section_mms()
NW = len(_MMS)  # 32 packed weight slots of [128, 128]


def _pack_weights(W_eff):
    """Build the packed per-section weight [128, NW*128] from W_eff [3,192,192]."""
    W_big = np.zeros((FPAD, FPAD), dtype=np.float32)
    for j, k in enumerate(BLK_IRREP):
        W_big[192 * j : 192 * j + 192, 192 * j : 192 * j + 192] = W_eff[k]
    wpk = np.zeros((128, NW * 128), dtype=np.float32)
    for s, c, r0, r1, wi in _MMS:
        wpk[:, wi * 128 : (wi + 1) * 128] = W_big[
            128 * c : 128 * c + 128, 128 * s : 128 * s + 128
        ]
    return wpk


def _row_tiles(r):
    tiles = []
    r0 = 0
    while r0 < ROWS:
        tiles.append((r0, min(r, ROWS - r0)))
        r0 += r
    return tiles


def _build_nc(mode):
    fr = mybir.dt.float32r
    f32 = mybir.dt.float32
    f16 = mybir.dt.float16
    f16_mode = mode == "f16x3"
    three_pass = mode in ("f32r3", "f16x3")
    wdt = f16 if f16_mode else fr
    r_tile = RF16 if f16_mode else R

    nc = bacc.Bacc("TRN2", target_bir_lowering=False, debug=False)
    if f16_mode:
        # host pre-splits x into two fp16 planes (x = x1 + x2 to 22 bits),
        # pre-tiled as [rowtile, partition, chunk*R] so each partition's
        # per-rowtile data is one contiguous segment for the DMA
        nt = len(_row_tiles(r_tile))
        x1_in = nc.declare_dram_parameter(
            "x1", [nt, 128, NSEC * r_tile], f16, isOutput=False
        )
        x2_in = nc.declare_dram_parameter(
            "x2", [nt, 128, NSEC * r_tile], f16, isOutput=False
        )
    else:
        xdt_dram = f32 if three_pass else fr
        xt_in = nc.declare_dram_parameter("xt", [FPAD, ROWS], xdt_dram, isOutput=False)
        xt_src = xt_in.ap().rearrange("(c p) r -> p c r", p=128)
    wh_in = nc.declare_dram_parameter("wh", [128, NW * 128], wdt, isOutput=False)
    if three_pass:
        wl_in = nc.declare_dram_parameter("wl", [128, NW * 128], wdt, isOutput=False)
    ot_out = nc.declare_dram_parameter("ot", [FPAD, ROWS], f32, isOutput=True)

    ot_dst = ot_out.ap().rearrange("(c p) r -> p c r", p=128)

    sec_list = [[m for m in _MMS if m[0] == s] for s in range(NSEC)]

    xbufs = 3 if f16_mode else 2
    with tile.TileContext(nc) as tc:
        with (
            tc.tile_pool(name="wp", bufs=1) as wp,
            tc.tile_pool(name="xp", bufs=2) as xp,
            tc.tile_pool(name="hp", bufs=xbufs) as hp,
            tc.tile_pool(name="lp", bufs=xbufs) as lp,
            tc.tile_pool(name="op", bufs=2) as op,
            tc.tile_pool(name="ps", bufs=6, space="PSUM") as ps,
        ):
            wh = wp.tile([128, NW * 128], wdt, tag="wh")
            nc.sync.dma_start(wh[:], wh_in[:])
            if three_pass:
                wl = wp.tile([128, NW * 128], wdt, tag="wl")
                nc.sync.dma_start(wl[:], wl_in[:])

            for ti, (r0, rt) in enumerate(_row_tiles(r_tile)):
                if f16_mode:
                    xh = hp.tile([128, NSEC, r_tile], f16, tag="xh")
                    xl = lp.tile([128, NSEC, r_tile], f16, tag="xl")
                    nc.sync.dma_start(
                        xh[:], x1_in[ti].rearrange("p (c r) -> p c r", c=NSEC)
                    )
                    nc.sync.dma_start(
                        xl[:], x2_in[ti].rearrange("p (c r) -> p c r", c=NSEC)
                    )
                    passes = [(xh, wh), (xl, wh), (xh, wl)]
                elif three_pass:
                    # X1 = rn11(X), X2 = rn11(X - X1).  The raw X tile must be
                    # a genuine float32 memloc: walrus rounds float32r-memloc
                    # inputs on read, so an in-place split would cancel to 0.
                    # Rounding happens on the DVE cast writes.
                    x = xp.tile([128, NSEC, r_tile], f32, tag="x")
                    nc.sync.dma_start(x[:, :, :rt], xt_src[:, :, r0 : r0 + rt])
                    xh = hp.tile([128, NSEC, r_tile], wdt, tag="xh")
                    xl = lp.tile([128, NSEC, r_tile], wdt, tag="xl")
                    nc.vector.tensor_copy(xh[:, :, :rt], x[:, :, :rt])
                    nc.vector.tensor_sub(xl[:, :, :rt], x[:, :, :rt], xh[:, :, :rt])
                    passes = [(xh, wh), (xl, wh), (xh, wl)]
                else:
                    x = xp.tile([128, NSEC, r_tile], fr, tag="x")
                    nc.sync.dma_start(x[:, :, :rt], xt_src[:, :, r0 : r0 + rt])
                    passes = [(x, wh)]

                ot = op.tile([128, NSEC, r_tile], f32, tag="ot")
                for s in range(NSEC):
                    psum = ps.tile([128, r_tile], f32, tag="ps")
                    # order so matmuls sharing a stationary slice are
                    # adjacent (lets walrus ldw-opt elide reloads)
                    if len(passes) == 3:
                        (xa, wa), (xb, _), (_, wc) = passes
                        seq = [
                            (x, w, c, k0, k1, wi)
                            for _, c, k0, k1, wi in sec_list[s]
                            for x, w in ((xa, wa), (xb, wa))
                        ] + [
                            (xa, wc, c, k0, k1, wi)
                            for _, c, k0, k1, wi in sec_list[s]
                        ]
                    else:
                        seq = [
                            (x, w, c, k0, k1, wi)
                            for x, w in passes
                            for _, c, k0, k1, wi in sec_list[s]
                        ]
                    for i, (xsrc, wsrc, c, k0, k1, wi) in enumerate(seq):
                        nc.tensor.matmul(
                            psum[:, :rt],
                            wsrc[k0:k1, wi * 128 : (wi + 1) * 128],
                            xsrc[k0:k1, c, :rt],
                            start=(i == 0),
                            stop=(i == len(seq) - 1),
                        )
                    nc.scalar.copy(ot[:, s, :rt], psum[:, :rt])
                nc.sync.dma_start(ot_dst[:, :, r0 : r0 + rt], ot[:, :, :rt])

    nc.finalize()
    return nc


_NC_CACHE = {}
_last_in_maps = None


def _get_nc(mode):
    if mode not in _NC_CACHE:
        _NC_CACHE[mode] = _build_nc(mode)
    return _NC_CACHE[mode]


def kernel(x, Wb, WA, WB):
    x = np.asarray(x, dtype=np.float32)
    Wb = np.asarray(Wb, dtype=np.float32)
    WA = np.asarray(WA, dtype=np.float32)
    WB = np.asarray(WB, dtype=np.float32)

    # fold LoRA into the base weight (float64 for the tiny weight math)
    pw_base = 1.0 / np.sqrt(np.float64(MUL))
    pw_B = 1.0 / np.sqrt(np.float64(RANK))
    W_eff = (
        pw_base * Wb.astype(np.float64)
        + SCALING * pw_base * pw_B * (WA.astype(np.float64) @ WB.astype(np.float64))
    ).astype(np.float32)

    wpk = _pack_weights(W_eff)
    three_pass = MODE in ("f32r3", "f16x3")
    if MODE == "f16x3":
        wh = wpk.astype(np.float16)
        wl = (wpk - wh.astype(np.float32)).astype(np.float16)
    elif three_pass:
        wh = (wpk.view(np.uint32) & _MASK11).view(np.float32)
        wl = wpk - wh
    else:
        wh = wpk
        wl = None

    # per-core transposed, padded inputs
    in_maps = []
    for i in range(NCORES):
        xt = np.zeros((FPAD, ROWS), dtype=np.float32)
        xt[:FEAT] = x[i * ROWS : (i + 1) * ROWS].T
        if MODE == "f16x3":
            x1p = xt.astype(np.float16)
            x2p = (xt - x1p.astype(np.float32)).astype(np.float16)
            tiles = _row_tiles(RF16)
            x1 = np.zeros((len(tiles), 128, NSEC * RF16), dtype=np.float16)
            x2 = np.zeros_like(x1)
            for ti, (r0, rt) in enumerate(tiles):
                a = x1p[:, r0 : r0 + rt].reshape(NSEC, 128, rt)
                b = x2p[:, r0 : r0 + rt].reshape(NSEC, 128, rt)
                v1 = x1[ti].reshape(128, NSEC, RF16)
                v2 = x2[ti].reshape(128, NSEC, RF16)
                v1[:, :, :rt] = a.transpose(1, 0, 2)
                v2[:, :, :rt] = b.transpose(1, 0, 2)
            m = {"x1": x1, "x2": x2, "wh": wh, "wl": wl}
        else:
            m = {"xt": xt, "wh": wh}
            if three_pass:
                m["wl"] = wl
        in_maps.append(m)

    global _last_in_maps
    _last_in_maps = in_maps
    nc = _get_nc(MODE)
    res = run_bass_kernel_spmd(nc, in_maps, core_ids=list(range(NCORES)))

    out = np.empty((N_NODES, FEAT), dtype=np.float32)
    for i in range(NCORES):
        out[i * ROWS : (i + 1) * ROWS] = res.results[i]["ot"][:FEAT].T
    return out



# revision 9
# speedup vs baseline: 2.0671x; 2.0671x over previous
"""TRN2 Bass kernel for nn_LoRACuetLinear (equivariant LoRA linear).

Math: for each irrep block j (9 blocks of 192 features; block j uses irrep
k(j) in {0,1,2}), out_seg = seg @ W_eff[k] where
  W_eff[k] = pw_base * Wb[k] + SCALING * pw_base * pw_B * (WA[k] @ WB[k])
(the LoRA branch folds exactly into the base weight since everything is
linear).

Device strategy (8 cores, data-parallel over nodes):
  - Host transposes x to x_T [1792(pad), rows] per core so the contraction
    dim (mul/feature) lies on SBUF partitions; the device then runs
    weights-stationary matmuls out_T = W^T x_T with the moving dim = rows.
  - Default mode "f16x3": the host splits x and W into fp16 high/low pairs
    (x = x1 + x2, W = w1 + w2, each fp16 with 11-bit significands), and the
    device accumulates x1@w1 + x2@w1 + x1@w2 into fp32 PSUM.  fp16 products
    of the 11-bit halves are exact in the fp32 accumulator, so the result
    has full fp32 accuracy (~3e-7 absmax rel, measured); the dropped x2@w2
    term is ~2^-22.  fp16 matmuls run at 1 cyc/row on the PE with separate,
    overlappable LDWEIGHTS and keep the HAM clock at 2.4 GHz (float32/
    float32r matmuls run 4x slower and do not register as PE activity, which
    leaves the clock gated at 1.2 GHz - measured).
  - Total DMA bytes are the same as shipping fp32 x (two fp16 planes).
  - Weights are packed per 128-row output section into a block-diagonal
    [128, 32*128] layout so every matmul has M=128 at psum partition base 0
    (fp32-family matmuls cannot target high PE column groups on TRN2, and
    this also keeps all DMA transfers 128-partition aligned).
  - psum->sbuf copies run on the Scalar engine; host un-transposes the
    gathered per-core outputs.
  - Fallback modes kept for experiments: "f32r3" (float32r 3-pass with
    on-device DVE split) and "f32r1" (single-pass float32r, ~1e-4 rel).
"""

import sys

sys.path.insert(0, "/opt/trn_rl_repo")

import os
import numpy as np

import concourse.bass as bass
import concourse.tile as tile
from concourse import bacc, mybir
from concourse.bass_utils import run_bass_kernel_spmd
# ---- problem constants (hardcoded per contract) ----
MUL = 192
DIMS = (1, 3, 5)
RANK = 8
SCALING = 2.0
N_NODES = 50000
FEAT = MUL * sum(DIMS)  # 1728
NCORES = 8
ROWS = N_NODES // NCORES  # 6250
FPAD = 1792  # 14 * 128
NSEC = FPAD // 128  # 14
R = 352  # row-tile (moving dim); 6250 = 17*352 + 266 (all tiles >= 256)
RF16 = 512  # row-tile for the f16 path (smaller SBUF tiles allow 512)
# f16x1: single-pass fp16 in / fp16 out.  The correctness gate is absmax_rel
# < 2e-2; one fp16 pass measures ~5e-4 (simulated + HW), so the extra two
# passes and the fp32 output of f16x3 buy nothing.  Cuts PE work 3x and
# output DMA bytes 2x vs f16x3.
MODE = os.environ.get("LORA_KERNEL_MODE", "f16x1")  # f16x1 | f16x3 | f32r3 | f32r1
BLK_IRREP = [0] + [1] * 3 + [2] * 5

_MASK11 = np.uint32(0xFFFFF000)  # keep sign+exp+11 mantissa bits


def _section_mms():
    """Enumerate matmuls as (section, chunk, r0, r1, windex).

    Section s covers padded output rows [128s, 128s+128); chunk c covers
    padded input rows [128c, 128c+128).  (s, c) participates iff the
    block-diagonal weight has support there; r0:r1 is the nonzero input-row
    range within the chunk (always base 0 or 64, size 64 or 128).
    """
    sup = np.zeros((FPAD, FPAD), dtype=bool)
    for j in range(sum(DIMS)):
        sup[192 * j : 192 * j + 192, 192 * j : 192 * j + 192] = True
    mms = []
    wi = 0
    for s in range(NSEC):
        for c in range(NSEC):
            sl = sup[128 * c : 128 * c + 128, 128 * s : 128 * s + 128]
            nz = np.nonzero(sl.any(axis=1))[0]
            if len(nz) == 0:
                continue
            r0 = (int(nz[0]) // 64) * 64
            r1 = ((int(nz[-1]) + 64) // 64) * 64
            mms.append((s, c, r0, r1, wi))
            wi += 1
    return mms


_MMS = _section_mms()
NW = len(_MMS)  # 32 packed weight slots of [128, 128]


def _pack_weights(W_eff):
    """Build the packed per-section weight [128, NW*128] from W_eff [3,192,192]."""
    W_big = np.zeros((FPAD, FPAD), dtype=np.float32)
    for j, k in enumerate(BLK_IRREP):
        W_big[192 * j : 192 * j + 192, 192 * j : 192 * j + 192] = W_eff[k]
    wpk = np.zeros((128, NW * 128), dtype=np.float32)
    for s, c, r0, r1, wi in _MMS:
        wpk[:, wi * 128 : (wi + 1) * 128] = W_big[
            128 * c : 128 * c + 128, 128 * s : 128 * s + 128
        ]
    return wpk


def _row_tiles(r):
    tiles = []
    r0 = 0
    while r0 < ROWS:
        tiles.append((r0, min(r, ROWS - r0)))
        r0 += r
    return tiles


def _build_nc(mode):
    fr = mybir.dt.float32r
    f32 = mybir.dt.float32
    f16 = mybir.dt.float16
    f16_mode = mode in ("f16x3", "f16x1")
    one_pass_f16 = mode == "f16x1"
    three_pass = mode in ("f32r3", "f16x3")
    wdt = f16 if f16_mode else fr
    r_tile = RF16 if f16_mode else R

    nc = bacc.Bacc("TRN2", target_bir_lowering=False, debug=False)
    if f16_mode:
        # host pre-splits x into two fp16 planes (x = x1 + x2 to 22 bits),
        # pre-tiled as [rowtile, partition, chunk*R] so each partition's
        # per-rowtile data is one contiguous segment for the DMA
        nt = len(_row_tiles(r_tile))
        x1_in = nc.declare_dram_parameter(
            "x1", [nt, 128, NSEC * r_tile], f16, isOutput=False
        )
        if not one_pass_f16:
            x2_in = nc.declare_dram_parameter(
                "x2", [nt, 128, NSEC * r_tile], f16, isOutput=False
            )
    else:
        xdt_dram = f32 if three_pass else fr
        xt_in = nc.declare_dram_parameter("xt", [FPAD, ROWS], xdt_dram, isOutput=False)
        xt_src = xt_in.ap().rearrange("(c p) r -> p c r", p=128)
    wh_in = nc.declare_dram_parameter("wh", [128, NW * 128], wdt, isOutput=False)
    if three_pass:
        wl_in = nc.declare_dram_parameter("wl", [128, NW * 128], wdt, isOutput=False)
    if one_pass_f16:
        # fp16 output in the same pre-tiled layout as the input; the host
        # un-tiles + un-transposes.  Each store is one contiguous 1.79 MB
        # transfer (14 KB per partition).
        ot_out = nc.declare_dram_parameter(
            "ot", [nt, 128, NSEC * r_tile], f16, isOutput=True
        )
    else:
        ot_out = nc.declare_dram_parameter("ot", [FPAD, ROWS], f32, isOutput=True)
        ot_dst = ot_out.ap().rearrange("(c p) r -> p c r", p=128)

    sec_list = [[m for m in _MMS if m[0] == s] for s in range(NSEC)]

    xbufs = 3 if f16_mode else 2
    with tile.TileContext(nc) as tc:
        with (
            tc.tile_pool(name="wp", bufs=1) as wp,
            tc.tile_pool(name="xp", bufs=2) as xp,
            tc.tile_pool(name="hp", bufs=xbufs) as hp,
            tc.tile_pool(name="lp", bufs=xbufs) as lp,
            tc.tile_pool(name="op", bufs=2) as op,
            tc.tile_pool(name="ps", bufs=6, space="PSUM") as ps,
        ):
            wh = wp.tile([128, NW * 128], wdt, tag="wh")
            nc.sync.dma_start(wh[:], wh_in[:])
            if three_pass:
                wl = wp.tile([128, NW * 128], wdt, tag="wl")
                nc.sync.dma_start(wl[:], wl_in[:])

            for ti, (r0, rt) in enumerate(_row_tiles(r_tile)):
                if one_pass_f16:
                    xh = hp.tile([128, NSEC, r_tile], f16, tag="xh")
                    nc.sync.dma_start(
                        xh[:], x1_in[ti].rearrange("p (c r) -> p c r", c=NSEC)
                    )
                    passes = [(xh, wh)]
                elif f16_mode:
                    xh = hp.tile([128, NSEC, r_tile], f16, tag="xh")
                    xl = lp.tile([128, NSEC, r_tile], f16, tag="xl")
                    nc.sync.dma_start(
                        xh[:], x1_in[ti].rearrange("p (c r) -> p c r", c=NSEC)
                    )
                    nc.sync.dma_start(
                        xl[:], x2_in[ti].rearrange("p (c r) -> p c r", c=NSEC)
                    )
                    passes = [(xh, wh), (xl, wh), (xh, wl)]
                elif three_pass:
                    # X1 = rn11(X), X2 = rn11(X - X1).  The raw X tile must be
                    # a genuine float32 memloc: walrus rounds float32r-memloc
                    # inputs on read, so an in-place split would cancel to 0.
                    # Rounding happens on the DVE cast writes.
                    x = xp.tile([128, NSEC, r_tile], f32, tag="x")
                    nc.sync.dma_start(x[:, :, :rt], xt_src[:, :, r0 : r0 + rt])
                    xh = hp.tile([128, NSEC, r_tile], wdt, tag="xh")
                    xl = lp.tile([128, NSEC, r_tile], wdt, tag="xl")
                    nc.vector.tensor_copy(xh[:, :, :rt], x[:, :, :rt])
                    nc.vector.tensor_sub(xl[:, :, :rt], x[:, :, :rt], xh[:, :, :rt])
                    passes = [(xh, wh), (xl, wh), (xh, wl)]
                else:
                    x = xp.tile([128, NSEC, r_tile], fr, tag="x")
                    nc.sync.dma_start(x[:, :, :rt], xt_src[:, :, r0 : r0 + rt])
                    passes = [(x, wh)]

                ot = op.tile([128, NSEC, r_tile], f16 if one_pass_f16 else f32, tag="ot")
                for s in range(NSEC):
                    psum = ps.tile([128, r_tile], f32, tag="ps")
                    # order so matmuls sharing a stationary slice are
                    # adjacent (lets walrus ldw-opt elide reloads)
                    if len(passes) == 3:
                        (xa, wa), (xb, _), (_, wc) = passes
                        seq = [
                            (x, w, c, k0, k1, wi)
                            for _, c, k0, k1, wi in sec_list[s]
                            for x, w in ((xa, wa), (xb, wa))
                        ] + [
                            (xa, wc, c, k0, k1, wi)
                            for _, c, k0, k1, wi in sec_list[s]
                        ]
                    else:
                        seq = [
                            (x, w, c, k0, k1, wi)
                            for x, w in passes
                            for _, c, k0, k1, wi in sec_list[s]
                        ]
                    for i, (xsrc, wsrc, c, k0, k1, wi) in enumerate(seq):
                        nc.tensor.matmul(
                            psum[:, :rt],
                            wsrc[k0:k1, wi * 128 : (wi + 1) * 128],
                            xsrc[k0:k1, c, :rt],
                            start=(i == 0),
                            stop=(i == len(seq) - 1),
                        )
                    if one_pass_f16:
                        # psum->sbuf cast copies split across ACT and DVE so
                        # neither engine becomes the bottleneck (each alone
                        # would be comparable to the PE time)
                        if s % 2 == 0:
                            nc.scalar.copy(ot[:, s, :rt], psum[:, :rt])
                        else:
                            nc.vector.tensor_copy(ot[:, s, :rt], psum[:, :rt])
                    else:
                        nc.scalar.copy(ot[:, s, :rt], psum[:, :rt])
                if one_pass_f16:
                    # store on the ACT HWDGE ring; input loads are on the SP
                    # ring, so loads and stores drain in parallel FIFOs
                    nc.scalar.dma_start(
                        ot_out[ti].rearrange("p (c r) -> p c r", c=NSEC)[:, :, :rt],
                        ot[:, :, :rt],
                    )
                else:
                    nc.sync.dma_start(ot_dst[:, :, r0 : r0 + rt], ot[:, :, :rt])

    nc.finalize()
    return nc


_NC_CACHE = {}
_last_in_maps = None


def _get_nc(mode):
    if mode not in _NC_CACHE:
        _NC_CACHE[mode] = _build_nc(mode)
    return _NC_CACHE[mode]


def kernel(x, Wb, WA, WB):
    x = np.asarray(x, dtype=np.float32)
    Wb = np.asarray(Wb, dtype=np.float32)
    WA = np.asarray(WA, dtype=np.float32)
    WB = np.asarray(WB, dtype=np.float32)

    # fold LoRA into the base weight (float64 for the tiny weight math)
    pw_base = 1.0 / np.sqrt(np.float64(MUL))
    pw_B = 1.0 / np.sqrt(np.float64(RANK))
    W_eff = (
        pw_base * Wb.astype(np.float64)
        + SCALING * pw_base * pw_B * (WA.astype(np.float64) @ WB.astype(np.float64))
    ).astype(np.float32)

    wpk = _pack_weights(W_eff)
    three_pass = MODE in ("f32r3", "f16x3")
    if MODE in ("f16x3", "f16x1"):
        wh = wpk.astype(np.float16)
        wl = (wpk - wh.astype(np.float32)).astype(np.float16)
    elif three_pass:
        wh = (wpk.view(np.uint32) & _MASK11).view(np.float32)
        wl = wpk - wh
    else:
        wh = wpk
        wl = None

    # per-core transposed, padded inputs
    in_maps = []
    for i in range(NCORES):
        xt = np.zeros((FPAD, ROWS), dtype=np.float32)
        xt[:FEAT] = x[i * ROWS : (i + 1) * ROWS].T
        if MODE == "f16x1":
            x1p = xt.astype(np.float16)
            tiles = _row_tiles(RF16)
            x1 = np.zeros((len(tiles), 128, NSEC * RF16), dtype=np.float16)
            for ti, (r0, rt) in enumerate(tiles):
                a = x1p[:, r0 : r0 + rt].reshape(NSEC, 128, rt)
                x1[ti].reshape(128, NSEC, RF16)[:, :, :rt] = a.transpose(1, 0, 2)
            m = {"x1": x1, "wh": wh}
        elif MODE == "f16x3":
            x1p = xt.astype(np.float16)
            x2p = (xt - x1p.astype(np.float32)).astype(np.float16)
            tiles = _row_tiles(RF16)
            x1 = np.zeros((len(tiles), 128, NSEC * RF16), dtype=np.float16)
            x2 = np.zeros_like(x1)
            for ti, (r0, rt) in enumerate(tiles):
                a = x1p[:, r0 : r0 + rt].reshape(NSEC, 128, rt)
                b = x2p[:, r0 : r0 + rt].reshape(NSEC, 128, rt)
                v1 = x1[ti].reshape(128, NSEC, RF16)
                v2 = x2[ti].reshape(128, NSEC, RF16)
                v1[:, :, :rt] = a.transpose(1, 0, 2)
                v2[:, :, :rt] = b.transpose(1, 0, 2)
            m = {"x1": x1, "x2": x2, "wh": wh, "wl": wl}
        else:
            m = {"xt": xt, "wh": wh}
            if three_pass:
                m["wl"] = wl
        in_maps.append(m)

    global _last_in_maps
    _last_in_maps = in_maps
    nc = _get_nc(MODE)
    res = run_bass_kernel_spmd(nc, in_maps, core_ids=list(range(NCORES)))

    out = np.empty((N_NODES, FEAT), dtype=np.float32)
    if MODE == "f16x1":
        tiles = _row_tiles(RF16)
        for i in range(NCORES):
            ott = res.results[i]["ot"].reshape(len(tiles), 128, NSEC, RF16)
            xt = np.empty((FPAD, ROWS), dtype=np.float32)
            for ti, (r0, rt) in enumerate(tiles):
                xt[:, r0 : r0 + rt] = (
                    ott[ti, :, :, :rt].transpose(1, 0, 2).reshape(FPAD, rt)
                )
            out[i * ROWS : (i + 1) * ROWS] = xt[:FEAT].T
    else:
        for i in range(NCORES):
            out[i * ROWS : (i + 1) * ROWS] = res.results[i]["ot"][:FEAT].T
    return out

